# revision 8
# baseline (speedup 1.0000x reference)
"""LoRA Linear kernel for Trainium2, 8 NeuronCores, data-parallel over tokens.

out = x @ W^T + bias + 2.0 * (x @ A^T) @ B^T
  x: [4, 2048, 4096] f32, W: [4096, 4096], bias: [4096], A: [16, 4096], B: [4096, 16]

Strategy (v2, fp8 DoubleRow):
  - Flatten tokens (8192) and shard 1024 tokens per core (pure data parallel,
    no collectives; gather on host).
  - Main GEMM in fp8e4 (e4m3, max 240) with perf_mode=DoubleRow: each matmul
    contracts K=256 (two 128-row planes packed per PE cell) at 0.5 cycles/row
    -> 4x fewer PE cycles than f32r. Host pre-scales x by 16 and W by 1024 so
    values sit mid-range; the 1/(16*1024) descale happens in the ACT-engine
    PSUM->SBUF evacuation (activation Copy with scale).
  - LoRA path stays bf16 for accuracy (fp8 x would push max rel err > 2e-2):
    xa^T = A x^T from a bf16 copy of x^T, then one K=128 bf16 matmul per
    output tile adds S*(2*B)^T @ xa^T AND S*bias (ones-row trick) into the
    same PSUM accumulation group before the descale.
  - Loop order k-pair-outer / m-chunk-inner: one DoubleRow LDWEIGHTS (256
    cols, ~213 ns) covers two N=512 matmuls (~214 ns) so weight loads hide.
  - W^T is host-packed per 128-wide output tile ([NO, P, KD, P]) so each
    per-oi DMA reads 4 KB contiguous per partition (fp8 would otherwise give
    128 B descriptors at ~half DMA efficiency).
  - Measured (CPU sim, full size): max rel err ~1.15e-2 vs fp32 reference
    (gate: 2e-2). bf16-everything fallback measures 3.6e-3.
"""

import sys
from contextlib import ExitStack

import numpy as np

sys.path.insert(0, "/opt/trn_rl_repo")

import concourse.bacc as bacc  # noqa: E402
import concourse.bass as bass  # noqa: E402
import concourse.mybir as mybir  # noqa: E402
import concourse.tile as tile  # noqa: E402
from concourse.bass import ts  # noqa: E402
from concourse.bass_utils import run_bass_kernel_spmd  # noqa: E402

P = 128
B_DIM, S_DIM = 4, 2048
D = 4096          # in_features (contraction)
O = 4096          # out_features
R = 16            # lora rank
SCALING = 2.0     # alpha / rank = 32/16
NCORES = 8
M = (B_DIM * S_DIM) // NCORES   # tokens per core = 1024
KD = D // P       # 32 contraction tiles of 128
KP = KD // 2      # 16 DoubleRow k-pairs (K=256 each)
MC = 512          # moving free dim per matmul
NMC = M // MC     # 2 m-chunks
NO = O // P       # 32 output-feature tiles

SX = 16.0         # x fp8 pre-scale
SW = 1024.0       # W fp8 pre-scale
S = SX * SW       # folded into ub rows host-side; undone at evacuation

FP8 = mybir.dt.float8e4
BF = mybir.dt.bfloat16
F32 = mybir.dt.float32
COPY = mybir.ActivationFunctionType.Copy


def build_program() -> bass.Bass:
    nc = bacc.Bacc()
    xtb = nc.dram_tensor("xtb", [D, M], BF, kind="ExternalInput")
    # W^T pre-packed per output tile: [oi, p, ko, o'] = W^T[ko*128+p, oi*128+o']
    wt8 = nc.dram_tensor("wt8", [NO, P, KD, P], FP8, kind="ExternalInput")
    atb = nc.dram_tensor("atb", [D, R], BF, kind="ExternalInput")
    # ubb: rows 0..15 = S*(2*lora_b)^T, row 16 = S*bias, rows 17..127 = 0
    ubb = nc.dram_tensor("ubb", [P, O], BF, kind="ExternalInput")
    # fill for xab rows 16..127: row 16 = ones, rest zeros
    fillb = nc.dram_tensor("fillb", [P - R, NMC, MC], BF, kind="ExternalInput")
    outT = nc.dram_tensor("outT", [O, M], BF, kind="ExternalOutput")

    xtb_r = xtb.rearrange("(ko p) m -> p ko m", p=P)   # [128, 32, 1024]
    atb_r = atb.rearrange("(ko p) r -> p ko r", p=P)   # [128, 32, 16]

    with ExitStack() as ctx:
        tc = ctx.enter_context(tile.TileContext(nc))
        xpool = ctx.enter_context(tc.tile_pool(name="xp", bufs=1))
        cpool = ctx.enter_context(tc.tile_pool(name="cpool", bufs=1))
        wt_pool = ctx.enter_context(tc.tile_pool(name="wtp", bufs=2))
        out_pool = ctx.enter_context(tc.tile_pool(name="outp", bufs=4))
        ps_pool = ctx.enter_context(tc.tile_pool(name="psp", bufs=2, space="PSUM"))
        psxa_pool = ctx.enter_context(tc.tile_pool(name="psxa", bufs=2, space="PSUM"))

        xt8_sb = xpool.tile([P, KD, M], FP8)
        xtb_sb = xpool.tile([P, KD, M], BF)
        at_sb = cpool.tile([P, KD, R], BF)
        ub_sb = cpool.tile([P, O], BF)            # rows 0..16 real, rest zero
        xab_sb = cpool.tile([P, NMC, MC], BF)     # rows 0..16 real, rest zero

        nc.scalar.dma_start(at_sb[:], atb_r)
        nc.scalar.dma_start(ub_sb[:], ubb[:])
        nc.scalar.dma_start(xab_sb[R:P, :, :], fillb[:])
        # x^T arrives once in bf16; the fp8 copy for the main GEMM is produced
        # on-chip by the (otherwise idle) DVE as each chunk lands, saving the
        # 4.2 MB fp8 HBM load and starting the main GEMM earlier.
        XSPLIT = 8
        kchunk = KD // XSPLIT
        for h in range(XSPLIT):
            nc.scalar.dma_start(
                xtb_sb[:, ts(h, kchunk), :], xtb_r[:, ts(h, kchunk), :]
            )
            nc.vector.tensor_scalar_mul(
                out=xt8_sb[:, ts(h, kchunk), :],
                in0=xtb_sb[:, ts(h, kchunk), :],
                scalar1=SX,
            )

        # xa^T[r, m] = sum_d A^T[d, r]^T x^T[d, m] for each m-chunk (bf16)
        for mi in range(NMC):
            ps_xa = psxa_pool.tile([R, MC], F32)
            for k in range(KD):
                nc.tensor.matmul(
                    ps_xa[:],
                    lhsT=at_sb[:, k, :],
                    rhs=xtb_sb[:, k, ts(mi, MC)],
                    start=(k == 0),
                    stop=(k == KD - 1),
                )
            nc.vector.tensor_copy(out=xab_sb[0:R, mi, :], in_=ps_xa[:])

        # Main: out^T tile [o=128, m=512]; 16 DoubleRow K=256 matmuls + one
        # bf16 K=128 matmul for S*(LoRA update + bias), then ACT descale.
        for oi in range(NO):
            wt_sb = wt_pool.tile([P, KD, P], FP8)
            nc.sync.dma_start(wt_sb[:], wt8[oi])
            ps = [ps_pool.tile([P, MC], F32, name=f"ps{mi}") for mi in range(NMC)]
            for c in range(KP):
                for mi in range(NMC):
                    nc.tensor.matmul(
                        ps[mi][:],
                        lhsT=wt_sb[:, ts(c, 2), :],
                        rhs=xt8_sb[:, ts(c, 2), ts(mi, MC)],
                        start=(c == 0),
                        stop=False,
                        perf_mode=mybir.MatmulPerfMode.DoubleRow,
                    )
            for mi in range(NMC):
                nc.tensor.matmul(
                    ps[mi][:],
                    lhsT=ub_sb[:, ts(oi, P)],
                    rhs=xab_sb[:, mi, :],
                    start=False,
                    stop=True,
                )
                ot = out_pool.tile([P, MC], F32)
                nc.scalar.activation(out=ot[:], in_=ps[mi][:], func=COPY, scale=1.0 / S)
                nc.gpsimd.dma_start(outT[ts(oi, P), ts(mi, MC)], ot[:])
    nc.compile()
    return nc


def prepare_in_maps(inputs, weight, bias, lora_a, lora_b):
    f8 = mybir.dt.np(FP8)
    bf16 = mybir.dt.np(BF)
    x = np.ascontiguousarray(
        np.asarray(inputs, dtype=np.float32).reshape(B_DIM * S_DIM, D)
    )
    wT = np.asarray(weight, dtype=np.float32).T                      # [D, O]
    wt8 = np.clip(wT * SW, -240.0, 240.0).astype(f8)                 # [D, O]
    # pack per output tile: [oi, p, ko, o'] with d = ko*128 + p
    wt8_packed = np.ascontiguousarray(
        wt8.reshape(KD, P, NO, P).transpose(2, 1, 0, 3)
    )
    atb = np.ascontiguousarray(np.asarray(lora_a, dtype=np.float32).T).astype(bf16)
    ubb = np.concatenate(
        [
            S * SCALING * np.asarray(lora_b, dtype=np.float32).T,
            S * np.asarray(bias, dtype=np.float32)[None, :],
            np.zeros((P - R - 1, O), dtype=np.float32),
        ],
        axis=0,
    ).astype(bf16)
    fillb = np.zeros((P - R, NMC, MC), dtype=np.float32)
    fillb[0] = 1.0
    fillb = fillb.astype(bf16)
    in_maps = []
    for c in range(NCORES):
        xt_c = np.ascontiguousarray(x[c * M : (c + 1) * M].T)        # [D, M]
        in_maps.append(
            {
                "xtb": xt_c.astype(bf16),
                "wt8": wt8_packed,
                "atb": atb,
                "ubb": ubb,
                "fillb": fillb,
            }
        )
    return in_maps


def run(inputs, weight, bias, lora_a, lora_b, trace=False):
    nc = build_program()
    in_maps = prepare_in_maps(inputs, weight, bias, lora_a, lora_b)
    res = run_bass_kernel_spmd(nc, in_maps, list(range(NCORES)), trace=trace)
    shards = [np.asarray(res.results[c]["outT"]).T for c in range(NCORES)]
    out = np.concatenate(shards, axis=0).reshape(B_DIM, S_DIM, O)
    return np.ascontiguousarray(out, dtype=np.float32), res


def kernel(inputs, weight, bias, lora_a, lora_b):
    out, _ = run(inputs, weight, bias, lora_a, lora_b, trace=False)
    return out


# revision 19
# speedup vs baseline: 16.8773x; 16.8773x over previous
"""LoRA Linear kernel for Trainium2, 8 NeuronCores, data-parallel over tokens.

out = x @ W^T + bias + 2.0 * (x @ A^T) @ B^T
  x: [4, 2048, 4096] f32, W: [4096, 4096], bias: [4096], A: [16, 4096], B: [4096, 16]

Strategy (v2, fp8 DoubleRow):
  - Flatten tokens (8192) and shard 1024 tokens per core (pure data parallel,
    no collectives; gather on host).
  - Main GEMM in fp8e4 (e4m3, max 240) with perf_mode=DoubleRow: each matmul
    contracts K=256 (two 128-row planes packed per PE cell) at 0.5 cycles/row
    -> 4x fewer PE cycles than f32r. Host pre-scales x by 16 and W by 1024 so
    values sit mid-range; the 1/(16*1024) descale happens in the ACT-engine
    PSUM->SBUF evacuation (activation Copy with scale).
  - LoRA path stays bf16 for accuracy (fp8 x would push max rel err > 2e-2):
    xa^T = A x^T from a bf16 copy of x^T, then one K=128 bf16 matmul per
    output tile adds S*(2*B)^T @ xa^T AND S*bias (ones-row trick) into the
    same PSUM accumulation group before the descale.
  - Loop order k-pair-outer / m-chunk-inner: one DoubleRow LDWEIGHTS (256
    cols, ~213 ns) covers two N=512 matmuls (~214 ns) so weight loads hide.
  - W^T is host-packed per 128-wide output tile ([NO, P, KD, P]) so each
    per-oi DMA reads 4 KB contiguous per partition (fp8 would otherwise give
    128 B descriptors at ~half DMA efficiency).
  - Output is written bf16 (half the store traffic; A/B-measured ~140 us
    faster than f32 out) and converted to f32 on host after the gather.
  - Measured: max rel err 1.167e-2 vs fp32 reference (gate: 2e-2);
    bf16-everything fallback would measure 3.6e-3.
"""

import sys
from contextlib import ExitStack

import numpy as np

sys.path.insert(0, "/opt/trn_rl_repo")

import concourse.bacc as bacc  # noqa: E402
import concourse.bass as bass  # noqa: E402
import concourse.mybir as mybir  # noqa: E402
import concourse.tile as tile  # noqa: E402
from concourse.bass import ts  # noqa: E402
from concourse.bass_utils import run_bass_kernel_spmd  # noqa: E402

P = 128
B_DIM, S_DIM = 4, 2048
D = 4096          # in_features (contraction)
O = 4096          # out_features
R = 16            # lora rank
SCALING = 2.0     # alpha / rank = 32/16
NCORES = 8
M = (B_DIM * S_DIM) // NCORES   # tokens per core = 1024
KD = D // P       # 32 contraction tiles of 128
KP = KD // 2      # 16 DoubleRow k-pairs (K=256 each)
MC = 512          # moving free dim per matmul
NMC = M // MC     # 2 m-chunks
NO = O // P       # 32 output-feature tiles

SX = 16.0         # x fp8 pre-scale
SW = 1024.0       # W fp8 pre-scale
S = SX * SW       # folded into ub rows host-side; undone at evacuation

FP8 = mybir.dt.float8e4
BF = mybir.dt.bfloat16
F32 = mybir.dt.float32
COPY = mybir.ActivationFunctionType.Copy


def build_program(
    xsplit: int = 4,
    out_merge: bool = False,
    ps_bufs: int = 2,
    wt_bufs: int = 2,
    out_ring=None,
) -> bass.Bass:
    nc = bacc.Bacc()
    xt8 = nc.dram_tensor("xt8", [D, M], FP8, kind="ExternalInput")
    xtb = nc.dram_tensor("xtb", [D, M], BF, kind="ExternalInput")
    # W^T pre-packed per output tile: [oi, p, ko, o'] = W^T[ko*128+p, oi*128+o']
    wt8 = nc.dram_tensor("wt8", [NO, P, KD, P], FP8, kind="ExternalInput")
    atb = nc.dram_tensor("atb", [D, R], BF, kind="ExternalInput")
    # ubb: rows 0..15 = S*(2*lora_b)^T, row 16 = S*bias, rows 17..127 = 0
    ubb = nc.dram_tensor("ubb", [P, O], BF, kind="ExternalInput")
    # fill for xab rows 16..127: row 16 = ones, rest zeros
    fillb = nc.dram_tensor("fillb", [P - R, NMC, MC], BF, kind="ExternalInput")
    outT = nc.dram_tensor("outT", [O, M], BF, kind="ExternalOutput")

    xt8_r = xt8.rearrange("(ko p) m -> p ko m", p=P)   # [128, 32, 1024]
    xtb_r = xtb.rearrange("(ko p) m -> p ko m", p=P)   # [128, 32, 1024]
    atb_r = atb.rearrange("(ko p) r -> p ko r", p=P)   # [128, 32, 16]

    with ExitStack() as ctx:
        tc = ctx.enter_context(tile.TileContext(nc))
        xpool = ctx.enter_context(tc.tile_pool(name="xp", bufs=1))
        cpool = ctx.enter_context(tc.tile_pool(name="cpool", bufs=1))
        wt_pool = ctx.enter_context(tc.tile_pool(name="wtp", bufs=wt_bufs))
        out_pool = ctx.enter_context(tc.tile_pool(name="outp", bufs=4))
        ps_pool = ctx.enter_context(tc.tile_pool(name="psp", bufs=ps_bufs, space="PSUM"))
        psxa_pool = ctx.enter_context(tc.tile_pool(name="psxa", bufs=2, space="PSUM"))

        xt8_sb = xpool.tile([P, KD, M], FP8)
        xtb_sb = xpool.tile([P, KD, M], BF)
        at_sb = cpool.tile([P, KD, R], BF)
        ub_sb = cpool.tile([P, O], BF)            # rows 0..16 real, rest zero
        xab_sb = cpool.tile([P, NMC, MC], BF)     # rows 0..16 real, rest zero

        nc.scalar.dma_start(at_sb[:], atb_r)
        nc.scalar.dma_start(ub_sb[:], ubb[:])
        nc.scalar.dma_start(xab_sb[R:P, :, :], fillb[:])
        # bf16 x^T first (feeds the xa prologue), fp8 x^T second (main GEMM
        # only completes a PSUM tile once the full contraction has arrived).
        # (An on-chip DVE bf16->fp8 conversion was tried instead of the dual
        # load; it regressed ~2x on HW despite simming fine — fp8 DVE output
        # appears far slower on silicon than the cost model thinks.)
        XSPLIT = xsplit
        kchunk = KD // XSPLIT
        for h in range(XSPLIT):
            nc.scalar.dma_start(
                xtb_sb[:, ts(h, kchunk), :], xtb_r[:, ts(h, kchunk), :]
            )
        for h in range(XSPLIT):
            nc.scalar.dma_start(
                xt8_sb[:, ts(h, kchunk), :], xt8_r[:, ts(h, kchunk), :]
            )

        # xa^T[r, m] = sum_d A^T[d, r]^T x^T[d, m] for each m-chunk (bf16)
        for mi in range(NMC):
            ps_xa = psxa_pool.tile([R, MC], F32)
            for k in range(KD):
                nc.tensor.matmul(
                    ps_xa[:],
                    lhsT=at_sb[:, k, :],
                    rhs=xtb_sb[:, k, ts(mi, MC)],
                    start=(k == 0),
                    stop=(k == KD - 1),
                )
            nc.vector.tensor_copy(out=xab_sb[0:R, mi, :], in_=ps_xa[:])

        # Main: out^T tile [o=128, m=512]; 16 DoubleRow K=256 matmuls + one
        # bf16 K=128 matmul for S*(LoRA update + bias), then ACT descale.
        for oi in range(NO):
            wt_sb = wt_pool.tile([P, KD, P], FP8)
            nc.sync.dma_start(wt_sb[:], wt8[oi])
            ps = [ps_pool.tile([P, MC], F32, name=f"ps{mi}") for mi in range(NMC)]
            for c in range(KP):
                for mi in range(NMC):
                    nc.tensor.matmul(
                        ps[mi][:],
                        lhsT=wt_sb[:, ts(c, 2), :],
                        rhs=xt8_sb[:, ts(c, 2), ts(mi, MC)],
                        start=(c == 0),
                        stop=False,
                        perf_mode=mybir.MatmulPerfMode.DoubleRow,
                    )
            ring = nc.gpsimd if out_ring is None else out_ring(nc)
            if out_merge:
                om = out_pool.tile([P, M], BF, name="om")
                for mi in range(NMC):
                    nc.tensor.matmul(
                        ps[mi][:],
                        lhsT=ub_sb[:, ts(oi, P)],
                        rhs=xab_sb[:, mi, :],
                        start=False,
                        stop=True,
                    )
                    nc.scalar.activation(
                        out=om[:, ts(mi, MC)], in_=ps[mi][:], func=COPY, scale=1.0 / S
                    )
                ring.dma_start(outT[ts(oi, P), :], om[:])
            else:
                for mi in range(NMC):
                    nc.tensor.matmul(
                        ps[mi][:],
                        lhsT=ub_sb[:, ts(oi, P)],
                        rhs=xab_sb[:, mi, :],
                        start=False,
                        stop=True,
                    )
                    ot = out_pool.tile([P, MC], F32, name="ot")
                    nc.scalar.activation(
                        out=ot[:], in_=ps[mi][:], func=COPY, scale=1.0 / S
                    )
                    ring.dma_start(outT[ts(oi, P), ts(mi, MC)], ot[:])
    nc.compile()
    return nc


def prepare_in_maps(inputs, weight, bias, lora_a, lora_b):
    f8 = mybir.dt.np(FP8)
    bf16 = mybir.dt.np(BF)
    x = np.ascontiguousarray(
        np.asarray(inputs, dtype=np.float32).reshape(B_DIM * S_DIM, D)
    )
    wT = np.asarray(weight, dtype=np.float32).T                      # [D, O]
    wt8 = np.clip(wT * SW, -240.0, 240.0).astype(f8)                 # [D, O]
    # pack per output tile: [oi, p, ko, o'] with d = ko*128 + p
    wt8_packed = np.ascontiguousarray(
        wt8.reshape(KD, P, NO, P).transpose(2, 1, 0, 3)
    )
    atb = np.ascontiguousarray(np.asarray(lora_a, dtype=np.float32).T).astype(bf16)
    ubb = np.concatenate(
        [
            S * SCALING * np.asarray(lora_b, dtype=np.float32).T,
            S * np.asarray(bias, dtype=np.float32)[None, :],
            np.zeros((P - R - 1, O), dtype=np.float32),
        ],
        axis=0,
    ).astype(bf16)
    fillb = np.zeros((P - R, NMC, MC), dtype=np.float32)
    fillb[0] = 1.0
    fillb = fillb.astype(bf16)
    in_maps = []
    for c in range(NCORES):
        xt_c = np.ascontiguousarray(x[c * M : (c + 1) * M].T)        # [D, M]
        in_maps.append(
            {
                "xt8": np.clip(xt_c * SX, -240.0, 240.0).astype(f8),
                "xtb": xt_c.astype(bf16),
                "wt8": wt8_packed,
                "atb": atb,
                "ubb": ubb,
                "fillb": fillb,
            }
        )
    return in_maps


def run(inputs, weight, bias, lora_a, lora_b, trace=False):
    nc = build_program()
    in_maps = prepare_in_maps(inputs, weight, bias, lora_a, lora_b)
    res = run_bass_kernel_spmd(nc, in_maps, list(range(NCORES)), trace=trace)
    shards = [np.asarray(res.results[c]["outT"]).T for c in range(NCORES)]
    out = np.concatenate(shards, axis=0).reshape(B_DIM, S_DIM, O)
    return np.ascontiguousarray(out, dtype=np.float32), res


def kernel(inputs, weight, bias, lora_a, lora_b):
    out, _ = run(inputs, weight, bias, lora_a, lora_b, trace=False)
    return out
